# revision 3
# baseline (speedup 1.0000x reference)
"""Trainium2 Bass kernel for the GNN coarsening layer (nn_Coarse_layer).

Pipeline (B=2 batches, N=100k nodes, E=800k edges, H=128, C=512 centroids):
  1. fine = relu(concat([x, dist]) @ W + b)                      [B,N,H]
  2. node_avg = segment_mean(fine, cci, C)                       [B,C,H]
  3. coarse edges: group edges by sorted centroid-pair key, drop
     intra-centroid edges, unique keys -> segment_mean(edge_attr) [B,Ec+1,H]

Strategy: host does the (cheap) integer index work — keys, unique, argsort,
greedy slot-aligned binning — and lays edge/node data out in DMA-friendly
order.  The 8 NeuronCores do all the heavy data movement and reduction:
segment sums are computed as one-hot matmuls on the tensor engine (one-hot
built on the vector engine from per-edge local ranks via iota/is_equal),
accumulated in PSUM, and streamed back.  Host then divides by counts and
scatters the compacted per-bin rows into the final output.
"""

import os
import sys

for _p in ("/opt/trn_rl_repo", "/root/.axon_site/_ro/trn_rl_repo"):
    if os.path.isdir(_p) and _p not in sys.path:
        sys.path.append(_p)

import numpy as np
import ml_dtypes

import concourse.bacc as bacc
import concourse.tile as tile
import concourse.mybir as mybir
from concourse.bass_utils import run_bass_kernel_spmd

F32 = mybir.dt.float32
BF16 = mybir.dt.bfloat16
NPBF16 = ml_dtypes.bfloat16

N_CENTROIDS = 512
E_COARSE = N_CENTROIDS * (N_CENTROIDS - 1) // 2
SENTINEL = N_CENTROIDS * N_CENTROIDS
N_CORES = 8

SUBT = 5                   # 128-edge subtiles per bin
CAP_EDGES = SUBT * 128     # max edges per bin (per batch)
MAX_SLOTS = 128            # max distinct coarse-edge slots per bin

# compute dtypes (bf16 halves the DMA traffic; sums still accumulate in fp32)
EDGE_BF16 = True           # edge_attr stream + edge one-hot
POOL_BF16 = True           # fine activations + node one-hot for pooling

LAST_EXEC_NS = None        # filled when BASS_KERNEL_TRACE=1


def _edge_prep(edge_index, cci):
    """Sort edges by coarse-pair slot and pack into slot-aligned bins."""
    sv = cci[edge_index[0]]
    ev = cci[edge_index[1]]
    vmin = np.minimum(sv, ev)
    vmax = np.maximum(sv, ev)
    keys = np.where(sv != ev, vmin * N_CENTROIDS + vmax, SENTINEL)
    uniq, inv = np.unique(keys, return_inverse=True)
    U = int(np.searchsorted(uniq, SENTINEL))        # valid (non-sentinel) slots
    order = np.argsort(inv, kind="stable")
    nval = int(np.searchsorted(inv[order], U))      # edges in valid slots
    order = order[:nval]
    sorted_inv = inv[order]
    counts = np.bincount(sorted_inv, minlength=max(U, 1))[:U]
    cum = np.zeros(U + 1, np.int64)
    cum[1:] = np.cumsum(counts)

    # greedy bins: consecutive slots, <= CAP_EDGES edges, <= MAX_SLOTS slots
    starts = [0]
    s = 0
    while s < U:
        e = int(np.searchsorted(cum, cum[s] + CAP_EDGES, side="right")) - 1
        e = min(e, s + MAX_SLOTS, U)
        if e <= s:
            raise RuntimeError(f"slot {s} has {counts[s]} edges > CAP {CAP_EDGES}")
        starts.append(e)
        s = e
    bin_lo = np.asarray(starts[:-1], np.int64)
    bin_hi = np.asarray(starts[1:], np.int64)
    n_bins = len(bin_lo)

    ar = np.arange(CAP_EDGES, dtype=np.int64)
    ofs = cum[bin_lo][:, None] + ar[None, :]             # [nb, CAP]
    ok = ofs < cum[bin_hi][:, None]
    ofs_c = np.minimum(ofs, max(nval - 1, 0))
    gath = np.where(ok, order[ofs_c], 0)                 # edge ids  [nb, CAP]
    ranks = np.where(ok, (sorted_inv[ofs_c] - bin_lo[:, None]).astype(np.float32),
                     np.float32(-1.0))                   # local slot [nb, CAP]
    return dict(uniq=uniq, U=U, counts=counts, bin_lo=bin_lo, bin_hi=bin_hi,
                n_bins=n_bins, gath=gath, ranks=ranks)


def _build_program(nbc, nt, edt, pdt):
    """One SPMD program; all per-core variation comes in through inputs."""
    nc = bacc.Bacc("TRN2", target_bir_lowering=False, debug=False,
                   enable_asserts=False, num_devices=N_CORES)
    ep_in = nc.dram_tensor("ep", [nbc, 128, SUBT * 256], edt, kind="ExternalInput").ap()
    rk_in = nc.dram_tensor("rk", [128, nbc * SUBT], F32, kind="ExternalInput").ap()
    xt_in = nc.dram_tensor("xt", [nt, 2, 128, 128], F32, kind="ExternalInput").ap()
    cc_in = nc.dram_tensor("cc", [128, nt], F32, kind="ExternalInput").ap()
    d1_in = nc.dram_tensor("d1", [2, nt * 128], F32, kind="ExternalInput").ap()
    w_in = nc.dram_tensor("wf", [130, 128], F32, kind="ExternalInput").ap()
    io_in = nc.dram_tensor("io", [128, 512], F32, kind="ExternalInput").ap()
    ie_in = nc.dram_tensor("ie", [128, 128], edt, kind="ExternalInput").ap()
    st_out = nc.dram_tensor("stage", [nbc, 128, 256], F32, kind="ExternalOutput").ap()
    pl_out = nc.dram_tensor("pool", [2, 128, 512], F32, kind="ExternalOutput").ap()

    relu = mybir.ActivationFunctionType.Relu
    iseq = mybir.AluOpType.is_equal

    with tile.TileContext(nc) as tc:
        with tc.tile_pool(name="consts", bufs=1) as consts, \
             tc.tile_pool(name="ep", bufs=3) as ep_pool, \
             tc.tile_pool(name="ohe", bufs=4) as ohe_pool, \
             tc.tile_pool(name="stg", bufs=3) as stg_pool, \
             tc.tile_pool(name="xp", bufs=4) as x_pool, \
             tc.tile_pool(name="fine", bufs=3) as fine_pool, \
             tc.tile_pool(name="ohn", bufs=3) as ohn_pool, \
             tc.tile_pool(name="pout", bufs=1) as pout_pool, \
             tc.tile_pool(name="eps", bufs=2, space="PSUM") as eps_pool, \
             tc.tile_pool(name="fps", bufs=2, space="PSUM") as fps_pool, \
             tc.tile_pool(name="pps", bufs=1, space="PSUM") as pps_pool:

            w128 = consts.tile([128, 128], F32, tag="w128")
            nc.sync.dma_start(w128[:], w_in[0:128])
            w2b = consts.tile([2, 128], F32, tag="w2b")
            nc.sync.dma_start(w2b[:], w_in[128:130])
            iot = consts.tile([128, 512], F32, tag="iot")
            nc.sync.dma_start(iot[:], io_in[:])
            iote = consts.tile([128, 128], edt, tag="iote")
            nc.sync.dma_start(iote[:], ie_in[:])
            cct = consts.tile([128, nt], F32, tag="cct")
            nc.sync.dma_start(cct[:], cc_in[:])
            d1t = consts.tile([2, nt * 128], F32, tag="d1t")
            nc.sync.dma_start(d1t[:], d1_in[:])
            rkt = consts.tile([128, nbc * SUBT], F32, tag="rkt")
            nc.sync.dma_start(rkt[:], rk_in[:])

            pool_ps = [pps_pool.tile([128, 512], F32, tag=f"pp{b}", name=f"pool_ps{b}")
                       for b in range(2)]

            def edge_bin(bb):
                ept = ep_pool.tile([128, SUBT * 256], edt)
                nc.sync.dma_start(ept[:], ep_in[bb])
                eps = eps_pool.tile([128, 256], F32)
                for j in range(SUBT):
                    ohe = ohe_pool.tile([128, 128], edt)
                    k = bb * SUBT + j
                    nc.vector.tensor_scalar(ohe[:], iote[:], rkt[:, k:k + 1], None, iseq)
                    nc.tensor.matmul(eps[:], ohe[:], ept[:, j * 256:(j + 1) * 256],
                                     start=(j == 0), stop=(j == SUBT - 1))
                stg = stg_pool.tile([128, 256], F32)
                nc.vector.tensor_copy(stg[:], eps[:])
                nc.sync.dma_start(st_out[bb], stg[:])

            def node_tile(t):
                ohn = ohn_pool.tile([128, 512], pdt)
                nc.vector.tensor_scalar(ohn[:], iot[:], cct[:, t:t + 1], None, iseq)
                for bch in range(2):
                    xtt = x_pool.tile([128, 128], F32)
                    nc.sync.dma_start(xtt[:], xt_in[t, bch])
                    fps = fps_pool.tile([128, 128], F32)
                    nc.tensor.matmul(fps[:], xtt[:], w128[:], start=True, stop=False)
                    nc.tensor.matmul(fps[:], d1t[:, t * 128:(t + 1) * 128], w2b[:],
                                     start=False, stop=True)
                    fsb = fine_pool.tile([128, 128], pdt, tag=f"fsb{bch}")
                    nc.scalar.activation(fsb[:], fps[:], relu)
                    nc.tensor.matmul(pool_ps[bch][:], fsb[:], ohn[:],
                                     start=(t == 0), stop=(t == nt - 1))

            for i in range(max(nbc, nt)):
                if i < nbc:
                    edge_bin(i)
                if i < nt:
                    node_tile(i)

            for bch in range(2):
                po = pout_pool.tile([128, 512], F32, tag=f"po{bch}")
                nc.vector.tensor_copy(po[:], pool_ps[bch][:])
                nc.sync.dma_start(pl_out[bch], po[:])

    nc.compile()
    return nc


def kernel(x, edge_index, edge_attr, scale=None, closest_centroid_indices=None,
           distances=None, W=None, b=None, **_unused):
    global LAST_EXEC_NS
    x_np = np.asarray(x, dtype=np.float32)
    ei = np.asarray(edge_index)
    ea = np.asarray(edge_attr, dtype=np.float32)
    cci_in = closest_centroid_indices
    cci = np.asarray(cci_in).astype(np.int64)
    dist = np.asarray(distances, dtype=np.float32)
    W_np = np.asarray(W, dtype=np.float32)
    b_np = np.asarray(b, dtype=np.float32)

    B, N, H = x_np.shape
    E = ei.shape[1]
    assert H == 128 and B == 2 and N % N_CORES == 0

    edt_np = NPBF16 if EDGE_BF16 else np.float32
    edt = BF16 if EDGE_BF16 else F32
    pdt = BF16 if POOL_BF16 else F32

    # ---------------- host index prep ----------------
    ep_meta = _edge_prep(ei.astype(np.int64), cci)
    n_bins = ep_meta["n_bins"]
    nbc = (n_bins + N_CORES - 1) // N_CORES
    npc = N // N_CORES
    nt = (npc + 127) // 128
    npc_pad = nt * 128

    # ---------------- per-core input maps ----------------
    iota512 = np.broadcast_to(np.arange(512, dtype=np.float32), (128, 512)).copy()
    iotae = np.broadcast_to(np.arange(128, dtype=np.float32), (128, 128)).astype(edt_np)
    wfull = np.concatenate([W_np, b_np[None, :]], 0).astype(np.float32)  # [130,128]

    in_maps = []
    for c in range(N_CORES):
        g_lo, g_hi = c * nbc, min((c + 1) * nbc, n_bins)
        nb_real = max(g_hi - g_lo, 0)
        gath = np.zeros((nbc, CAP_EDGES), np.int64)
        ranks = np.full((nbc, CAP_EDGES), -1.0, np.float32)
        if nb_real > 0:
            gath[:nb_real] = ep_meta["gath"][g_lo:g_hi]
            ranks[:nb_real] = ep_meta["ranks"][g_lo:g_hi]
        # ep: [nbc, 128, SUBT*256]; column layout j*256 + batch*128 + h
        g = ea[:, gath.reshape(-1), :]                       # [2, nbc*CAP, 128]
        g = g.reshape(2, nbc, SUBT, 128, 128).transpose(1, 3, 2, 0, 4)
        ep = np.ascontiguousarray(g.reshape(nbc, 128, SUBT * 256)).astype(edt_np)
        rk = np.ascontiguousarray(
            ranks.reshape(nbc, SUBT, 128).transpose(2, 0, 1).reshape(128, nbc * SUBT)
        ).astype(np.float32)

        lo = c * npc
        xs = np.zeros((2, npc_pad, 128), np.float32)
        xs[:, :npc] = x_np[:, lo:lo + npc]
        xt = np.ascontiguousarray(
            xs.reshape(2, nt, 128, 128).transpose(1, 0, 3, 2))  # [nt,2,c,n]
        cc_pad = np.full(npc_pad, -1.0, np.float32)
        cc_pad[:npc] = cci[lo:lo + npc].astype(np.float32)
        cc = np.ascontiguousarray(cc_pad.reshape(nt, 128).T)    # [128, nt]
        d1 = np.ones((2, npc_pad), np.float32)
        d1[0, :] = 0.0
        d1[0, :npc] = dist[lo:lo + npc]

        in_maps.append({"ep": ep, "rk": rk, "xt": xt, "cc": cc, "d1": d1,
                        "wf": wfull, "io": iota512, "ie": iotae})

    # ---------------- build + run ----------------
    nc = _build_program(nbc, nt, edt, pdt)
    trace = os.environ.get("BASS_KERNEL_TRACE", "0") == "1"
    if trace:
        try:
            import profhook
            profhook.install()
        except Exception:
            trace = False
    res = run_bass_kernel_spmd(nc, in_maps, core_ids=list(range(N_CORES)),
                               trace=trace)
    LAST_EXEC_NS = res.exec_time_ns

    # ---------------- host postprocessing ----------------
    # node pooling: sum partial [h, s] pools, divide by counts
    pool_sum = np.zeros((2, 128, 512), np.float64)
    for c in range(N_CORES):
        pool_sum += res.results[c]["pool"]
    pool_sum = pool_sum.astype(np.float32)
    ncnt = np.bincount(cci, minlength=N_CENTROIDS).astype(np.float32)
    node_avg = pool_sum.transpose(0, 2, 1) / np.clip(ncnt, 1.0, None)[None, :, None]

    # coarse edge attrs
    U = ep_meta["U"]
    bin_lo = ep_meta["bin_lo"]
    rows = np.concatenate([res.results[c]["stage"].reshape(-1, 256)
                           for c in range(N_CORES)], 0)
    u_arr = np.arange(U)
    g_of_u = np.searchsorted(bin_lo, u_arr, side="right") - 1
    row_idx = g_of_u * 128 + (u_arr - bin_lo[g_of_u])
    esums = rows[row_idx]                                     # [U, 256]
    emeans = esums / np.clip(ep_meta["counts"].astype(np.float32), 1.0, None)[:, None]
    out_attr = np.zeros((2, E_COARSE + 1, 128), np.float32)
    out_attr[0, :U] = emeans[:, :128]
    out_attr[1, :U] = emeans[:, 128:]

    # coarse edge index
    uniq = ep_meta["uniq"]
    uniq_pad = np.full(E_COARSE + 1, SENTINEL, np.int64)
    uniq_pad[:min(len(uniq), E_COARSE + 1)] = uniq[:E_COARSE + 1]
    idx_dt = np.asarray(cci_in).dtype
    if idx_dt.kind not in "iu":
        idx_dt = np.dtype(np.int32)
    ce = np.where(uniq_pad < SENTINEL,
                  np.stack([uniq_pad // N_CENTROIDS, uniq_pad % N_CENTROIDS]),
                  -1).astype(idx_dt)

    return (node_avg, out_attr, ce, np.asarray(cci_in), np.asarray(distances))


# revision 7
# speedup vs baseline: 1.0957x; 1.0957x over previous
"""Trainium2 Bass kernel for the GNN coarsening layer (nn_Coarse_layer).

Pipeline (B=2 batches, N=100k nodes, E=800k edges, H=128, C=512 centroids):
  1. fine = relu(concat([x, dist]) @ W + b)                      [B,N,H]
  2. node_avg = segment_mean(fine, cci, C)                       [B,C,H]
  3. coarse edges: group edges by sorted centroid-pair key, drop
     intra-centroid edges, unique keys -> segment_mean(edge_attr) [B,Ec+1,H]

Strategy: the host does the cheap integer index work — pair keys, unique,
argsort — and pads each coarse-edge group to a multiple of PAD=4 edges
(null edges point at an appended all-zero row).  With that layout the big
segment-sum becomes a constant-stationary matmul: a fixed aggregation
matrix A^T[128, 32] (A[s, e] = 1 iff e//4 == s) reduces 128 streamed edge
rows into 32 partial-group rows per matmul, both batches side by side in
the moving operand.  The tensor engine streams with zero per-tile setup
work, PSUM accumulates in fp32, and the host divides by group counts and
sums the split sub-groups (np.add.reduceat) while assembling the output.
The tiny MLP + 512-way node pooling run on-device as bf16 matmuls with a
per-tile iota/is_equal one-hot for the pooling reduction.
"""

import os
import sys

for _p in ("/opt/trn_rl_repo", "/root/.axon_site/_ro/trn_rl_repo"):
    if os.path.isdir(_p) and _p not in sys.path:
        sys.path.append(_p)

import numpy as np
import ml_dtypes

import concourse.bacc as bacc
import concourse.tile as tile
import concourse.mybir as mybir
from concourse.bass_utils import run_bass_kernel_spmd

F32 = mybir.dt.float32
BF16 = mybir.dt.bfloat16
NPBF16 = ml_dtypes.bfloat16

N_CENTROIDS = 512
E_COARSE = N_CENTROIDS * (N_CENTROIDS - 1) // 2
SENTINEL = N_CENTROIDS * N_CENTROIDS
N_CORES = 8

PAD = 4                  # edges per sub-group (aggregation matrix row width)
SUBS = 128 // PAD        # sub-groups reduced per matmul (32)
STAGE_F32 = True         # staging rows dtype (f32 keeps sums exact)

LAST_EXEC_NS = None      # filled when BASS_KERNEL_TRACE=1


def _edge_prep(edge_index, cci, E):
    """Sort edges by coarse-pair slot; pad each slot to a multiple of PAD."""
    sv = cci[edge_index[0]]
    ev = cci[edge_index[1]]
    vmin = np.minimum(sv, ev)
    vmax = np.maximum(sv, ev)
    keys = np.where(sv != ev, vmin * N_CENTROIDS + vmax, SENTINEL)
    uniq, inv = np.unique(keys, return_inverse=True)
    U = int(np.searchsorted(uniq, SENTINEL))        # valid (non-sentinel) slots
    order = np.argsort(inv, kind="stable")
    nval = int(np.searchsorted(inv[order], U))      # edges in valid slots
    order = order[:nval]
    counts = np.bincount(inv[order], minlength=max(U, 1))[:U]
    cum = np.zeros(U + 1, np.int64)
    cum[1:] = np.cumsum(counts)

    subcnt = (counts + PAD - 1) // PAD
    sub_lo = np.zeros(U + 1, np.int64)
    sub_lo[1:] = np.cumsum(subcnt)
    S_sub = int(sub_lo[-1])
    slot_of_sub = np.repeat(np.arange(U, dtype=np.int64), subcnt)
    within = np.arange(S_sub, dtype=np.int64) - sub_lo[slot_of_sub]
    epos = cum[slot_of_sub][:, None] + within[:, None] * PAD + np.arange(PAD)[None, :]
    ok = epos < cum[slot_of_sub + 1][:, None]
    epos_c = np.minimum(epos, max(nval - 1, 0))
    gath = np.where(ok, order[epos_c], E)           # E == appended zero row
    return dict(uniq=uniq, U=U, counts=counts, sub_lo=sub_lo, S_sub=S_sub,
                gath=gath)


def _build_program(nsb, ntp, nt):
    """One SPMD program; all per-core variation comes in through inputs."""
    nc = bacc.Bacc("TRN2", target_bir_lowering=False, debug=False,
                   enable_asserts=False, num_devices=N_CORES)
    stage_dt = F32 if STAGE_F32 else BF16
    ep_in = nc.dram_tensor("ep", [nsb, 128, PAD * 256], BF16, kind="ExternalInput").ap()
    ag_in = nc.dram_tensor("ag", [128, SUBS], BF16, kind="ExternalInput").ap()
    xt_in = nc.dram_tensor("xt", [ntp, 2, 128, 256], BF16, kind="ExternalInput").ap()
    cc_in = nc.dram_tensor("cc", [128, nt], F32, kind="ExternalInput").ap()
    d1_in = nc.dram_tensor("d1", [2, nt * 128], BF16, kind="ExternalInput").ap()
    w_in = nc.dram_tensor("wf", [130, 128], BF16, kind="ExternalInput").ap()
    io_in = nc.dram_tensor("io", [128, 512], F32, kind="ExternalInput").ap()
    st_out = nc.dram_tensor("stage", [nsb, 128, 256], stage_dt, kind="ExternalOutput").ap()
    pl_out = nc.dram_tensor("pool", [2, 128, 512], F32, kind="ExternalOutput").ap()

    relu = mybir.ActivationFunctionType.Relu
    iseq = mybir.AluOpType.is_equal

    with tile.TileContext(nc) as tc:
        with tc.tile_pool(name="consts", bufs=1) as consts, \
             tc.tile_pool(name="ep", bufs=4) as ep_pool, \
             tc.tile_pool(name="stg", bufs=4) as stg_pool, \
             tc.tile_pool(name="xp", bufs=4) as x_pool, \
             tc.tile_pool(name="fine", bufs=3) as fine_pool, \
             tc.tile_pool(name="ohn", bufs=3) as ohn_pool, \
             tc.tile_pool(name="pout", bufs=1) as pout_pool, \
             tc.tile_pool(name="eps", bufs=2, space="PSUM") as eps_pool, \
             tc.tile_pool(name="fps", bufs=2, space="PSUM") as fps_pool, \
             tc.tile_pool(name="pps", bufs=1, space="PSUM") as pps_pool:

            agt = consts.tile([128, SUBS], BF16, tag="agt")
            nc.sync.dma_start(agt[:], ag_in[:])
            w128 = consts.tile([128, 128], BF16, tag="w128")
            nc.sync.dma_start(w128[:], w_in[0:128])
            w2b = consts.tile([2, 128], BF16, tag="w2b")
            nc.sync.dma_start(w2b[:], w_in[128:130])
            iot = consts.tile([128, 512], F32, tag="iot")
            nc.sync.dma_start(iot[:], io_in[:])
            cct = consts.tile([128, nt], F32, tag="cct")
            nc.sync.dma_start(cct[:], cc_in[:])
            d1t = consts.tile([2, nt * 128], BF16, tag="d1t")
            nc.sync.dma_start(d1t[:], d1_in[:])

            pool_ps = [pps_pool.tile([128, 512], F32, tag=f"pp{b}", name=f"pool_ps{b}")
                       for b in range(2)]

            def edge_superbin(sb):
                ept = ep_pool.tile([128, PAD * 256], BF16, tag="ept", name=f"ept{sb}")
                nc.sync.dma_start(ept[:], ep_in[sb])
                # matmul dst base partition must be 0/32/64 -> two 64-row tiles
                eps_t = [eps_pool.tile([64, 256], F32, tag=f"eps{hh}",
                                       name=f"eps{hh}_{sb}") for hh in range(2)]
                for j in range(PAD):
                    hh, off = divmod(j, 2)
                    nc.tensor.matmul(eps_t[hh][off * SUBS:(off + 1) * SUBS, :],
                                     agt[:], ept[:, j * 256:(j + 1) * 256],
                                     start=True, stop=True, skip_group_check=True)
                stg = stg_pool.tile([128, 256], F32 if STAGE_F32 else BF16,
                                    tag="stg", name=f"stg{sb}")
                nc.vector.tensor_copy(stg[0:64, :], eps_t[0][:])
                nc.vector.tensor_copy(stg[64:128, :], eps_t[1][:])
                nc.sync.dma_start(st_out[sb], stg[:])

            def node_pair(tp):
                xts = []
                for bch in range(2):
                    xtt = x_pool.tile([128, 256], BF16, tag=f"xt{bch}",
                                      name=f"xt{bch}_{tp}")
                    nc.sync.dma_start(xtt[:], xt_in[tp, bch])
                    xts.append(xtt)
                for half in range(2):
                    t = tp * 2 + half
                    ohn = ohn_pool.tile([128, 512], BF16, tag="ohn", name=f"ohn{t}")
                    nc.vector.tensor_scalar(ohn[:], iot[:], cct[:, t:t + 1], None, iseq)
                    for bch in range(2):
                        fps = fps_pool.tile([128, 128], F32, tag="fps", name=f"fps{t}_{bch}")
                        nc.tensor.matmul(fps[:], xts[bch][:, half * 128:(half + 1) * 128],
                                         w128[:], start=True, stop=False)
                        nc.tensor.matmul(fps[:], d1t[:, t * 128:(t + 1) * 128], w2b[:],
                                         start=False, stop=True)
                        fsb = fine_pool.tile([128, 128], BF16, tag=f"fsb{bch}",
                                             name=f"fsb{t}_{bch}")
                        nc.scalar.activation(fsb[:], fps[:], relu)
                        nc.tensor.matmul(pool_ps[bch][:], fsb[:], ohn[:],
                                         start=(t == 0), stop=(t == nt - 1))

            for i in range(max(nsb, ntp)):
                if i < nsb:
                    edge_superbin(i)
                if i < ntp:
                    node_pair(i)

            for bch in range(2):
                po = pout_pool.tile([128, 512], F32, tag=f"po{bch}", name=f"po{bch}")
                nc.vector.tensor_copy(po[:], pool_ps[bch][:])
                nc.sync.dma_start(pl_out[bch], po[:])

    nc.compile()
    return nc


def kernel(x, edge_index, edge_attr, scale=None, closest_centroid_indices=None,
           distances=None, W=None, b=None, **_unused):
    global LAST_EXEC_NS
    x_np = np.asarray(x, dtype=np.float32)
    ei = np.asarray(edge_index)
    ea = np.asarray(edge_attr, dtype=np.float32)
    cci_in = closest_centroid_indices
    cci = np.asarray(cci_in).astype(np.int64)
    dist = np.asarray(distances, dtype=np.float32)
    W_np = np.asarray(W, dtype=np.float32)
    b_np = np.asarray(b, dtype=np.float32)

    B, N, H = x_np.shape
    E = ei.shape[1]
    assert H == 128 and B == 2 and N % N_CORES == 0

    # ---------------- host index prep ----------------
    em = _edge_prep(ei.astype(np.int64), cci, E)
    S_sub = em["S_sub"]
    nsb = max((S_sub + N_CORES * 128 - 1) // (N_CORES * 128), 1)
    spc = nsb * 128                                    # sub-groups per core
    npc = N // N_CORES
    nt = (npc + 127) // 128
    if nt % 2:
        nt += 1
    ntp = nt // 2
    npc_pad = nt * 128

    ea_ext = np.concatenate([ea, np.zeros((2, 1, H), np.float32)], 1)

    iota512 = np.broadcast_to(np.arange(512, dtype=np.float32), (128, 512)).copy()
    aseg = (np.arange(128)[:, None] // PAD ==
            np.arange(SUBS)[None, :]).astype(NPBF16)
    wfull = np.concatenate([W_np, b_np[None, :]], 0).astype(NPBF16)  # [130,128]

    in_maps = []
    for c in range(N_CORES):
        lo_s, hi_s = c * spc, min((c + 1) * spc, S_sub)
        gath = np.full((spc, PAD), E, np.int64)
        if hi_s > lo_s:
            gath[:hi_s - lo_s] = em["gath"][lo_s:hi_s]
        g = ea_ext[:, gath.reshape(-1), :]              # [2, spc*PAD, 128]
        g = g.reshape(2, nsb, PAD, SUBS, PAD, 128).transpose(1, 3, 4, 2, 0, 5)
        ep = np.ascontiguousarray(g.reshape(nsb, 128, PAD * 256)).astype(NPBF16)

        lo = c * npc
        xs = np.zeros((2, npc_pad, 128), np.float32)
        xs[:, :npc] = x_np[:, lo:lo + npc]
        xt = np.ascontiguousarray(
            xs.reshape(2, ntp, 256, 128).transpose(1, 0, 3, 2)).astype(NPBF16)
        cc_pad = np.full(npc_pad, -1.0, np.float32)
        cc_pad[:npc] = cci[lo:lo + npc].astype(np.float32)
        cc = np.ascontiguousarray(cc_pad.reshape(nt, 128).T)    # [128, nt]
        d1 = np.ones((2, npc_pad), np.float32)
        d1[0, :] = 0.0
        d1[0, :npc] = dist[lo:lo + npc]

        in_maps.append({"ep": ep, "ag": aseg, "xt": xt, "cc": cc,
                        "d1": d1.astype(NPBF16), "wf": wfull, "io": iota512})

    # ---------------- build + run ----------------
    nc = _build_program(nsb, ntp, nt)
    trace = os.environ.get("BASS_KERNEL_TRACE", "0") == "1"
    if trace:
        try:
            import profhook
            profhook.install()
        except Exception:
            trace = False
    res = run_bass_kernel_spmd(nc, in_maps, core_ids=list(range(N_CORES)),
                               trace=trace)
    LAST_EXEC_NS = res.exec_time_ns

    # ---------------- host postprocessing ----------------
    pool_sum = np.zeros((2, 128, 512), np.float64)
    for c in range(N_CORES):
        pool_sum += res.results[c]["pool"]
    pool_sum = pool_sum.astype(np.float32)
    ncnt = np.bincount(cci, minlength=N_CENTROIDS).astype(np.float32)
    node_avg = pool_sum.transpose(0, 2, 1) / np.clip(ncnt, 1.0, None)[None, :, None]

    # coarse edge attrs: gather real sub-group rows, reduce per slot
    U = em["U"]
    chunks = []
    for c in range(N_CORES):
        lo_s, hi_s = c * spc, min((c + 1) * spc, S_sub)
        if hi_s > lo_s:
            chunks.append(res.results[c]["stage"].reshape(spc, 256)[:hi_s - lo_s])
    out_attr = np.zeros((2, E_COARSE + 1, 128), np.float32)
    if U > 0:
        rows = np.concatenate(chunks, 0).astype(np.float32)      # [S_sub, 256]
        esums = np.add.reduceat(rows, em["sub_lo"][:U], axis=0)  # [U, 256]
        emeans = esums / np.clip(em["counts"].astype(np.float32), 1.0, None)[:, None]
        out_attr[0, :U] = emeans[:, :128]
        out_attr[1, :U] = emeans[:, 128:]

    uniq = em["uniq"]
    uniq_pad = np.full(E_COARSE + 1, SENTINEL, np.int64)
    uniq_pad[:min(len(uniq), E_COARSE + 1)] = uniq[:E_COARSE + 1]
    idx_dt = np.asarray(cci_in).dtype
    if idx_dt.kind not in "iu":
        idx_dt = np.dtype(np.int32)
    ce = np.where(uniq_pad < SENTINEL,
                  np.stack([uniq_pad // N_CENTROIDS, uniq_pad % N_CENTROIDS]),
                  -1).astype(idx_dt)

    return (node_avg, out_attr, ce, np.asarray(cci_in), np.asarray(distances))


# revision 9
# speedup vs baseline: 1.6934x; 1.5455x over previous
"""Trainium2 Bass kernel for the GNN coarsening layer (nn_Coarse_layer).

Pipeline (B=2 batches, N=100k nodes, E=800k edges, H=128, C=512 centroids):
  1. fine = relu(concat([x, dist]) @ W + b)                      [B,N,H]
  2. node_avg = segment_mean(fine, cci, C)                       [B,C,H]
  3. coarse edges: group edges by sorted centroid-pair key, drop
     intra-centroid edges, unique keys -> segment_mean(edge_attr) [B,Ec+1,H]

Strategy: the host does the cheap integer index work — pair keys, unique,
argsort — and pads each coarse-edge group to a multiple of PAD=4 edges
(null edges point at an appended all-zero row).  With that layout the big
segment-sum becomes a constant-stationary matmul: fixed shifted aggregation
matrices A_j^T[128, 128] (A_j[s, e] = 1 iff s == j*32 + e//4) reduce 128
streamed edge rows into 32 partial-group rows each, all four subtiles
accumulating into one PSUM tile, with both batches side by side in the
moving operand.  The tensor engine streams with zero per-tile setup work,
PSUM accumulates in fp32, and the host divides by group counts and sums the
split sub-groups (np.add.reduceat) while assembling the output.  The tiny
MLP + 512-way node pooling run on-device as bf16 matmuls with a per-tile
iota/is_equal one-hot for the pooling reduction.  DMA traffic is batched:
GRP=4 superbins ride one 1MB load and one staging store.
"""

import os
import sys

for _p in ("/opt/trn_rl_repo", "/root/.axon_site/_ro/trn_rl_repo"):
    if os.path.isdir(_p) and _p not in sys.path:
        sys.path.append(_p)

import numpy as np
import ml_dtypes

import concourse.bacc as bacc
import concourse.tile as tile
import concourse.mybir as mybir
from concourse.bass_utils import run_bass_kernel_spmd

F32 = mybir.dt.float32
BF16 = mybir.dt.bfloat16
NPBF16 = ml_dtypes.bfloat16

N_CENTROIDS = 512
E_COARSE = N_CENTROIDS * (N_CENTROIDS - 1) // 2
SENTINEL = N_CENTROIDS * N_CENTROIDS
N_CORES = 8

PAD = 4                  # edges per sub-group (aggregation matrix row width)
SUBS = 128 // PAD        # sub-groups produced per matmul (32)
GRP = 4                  # superbins (128 sub-groups each) per DMA transfer
STAGE_F32 = False        # staging dtype; bf16 halves the store traffic

LAST_EXEC_NS = None      # filled when BASS_KERNEL_TRACE=1


def _edge_prep(edge_index, cci, E):
    """Sort edges by coarse-pair slot; pad each slot to a multiple of PAD."""
    sv = cci[edge_index[0]]
    ev = cci[edge_index[1]]
    vmin = np.minimum(sv, ev)
    vmax = np.maximum(sv, ev)
    keys = np.where(sv != ev, vmin * N_CENTROIDS + vmax, SENTINEL)
    uniq, inv = np.unique(keys, return_inverse=True)
    U = int(np.searchsorted(uniq, SENTINEL))        # valid (non-sentinel) slots
    order = np.argsort(inv, kind="stable")
    nval = int(np.searchsorted(inv[order], U))      # edges in valid slots
    order = order[:nval]
    counts = np.bincount(inv[order], minlength=max(U, 1))[:U]
    cum = np.zeros(U + 1, np.int64)
    cum[1:] = np.cumsum(counts)

    subcnt = (counts + PAD - 1) // PAD
    sub_lo = np.zeros(U + 1, np.int64)
    sub_lo[1:] = np.cumsum(subcnt)
    S_sub = int(sub_lo[-1])
    slot_of_sub = np.repeat(np.arange(U, dtype=np.int64), subcnt)
    within = np.arange(S_sub, dtype=np.int64) - sub_lo[slot_of_sub]
    epos = cum[slot_of_sub][:, None] + within[:, None] * PAD + np.arange(PAD)[None, :]
    ok = epos < cum[slot_of_sub + 1][:, None]
    epos_c = np.minimum(epos, max(nval - 1, 0))
    gath = np.where(ok, order[epos_c], E)           # E == appended zero row
    return dict(uniq=uniq, U=U, counts=counts, sub_lo=sub_lo, S_sub=S_sub,
                gath=gath)


def _build_program(nsg, ntp, nt):
    """One SPMD program; all per-core variation comes in through inputs."""
    nc = bacc.Bacc("TRN2", target_bir_lowering=False, debug=False,
                   enable_asserts=False, num_devices=N_CORES)
    stage_dt = F32 if STAGE_F32 else BF16
    ep_in = nc.dram_tensor("ep", [nsg, 128, GRP * 1024], BF16, kind="ExternalInput").ap()
    ag_in = nc.dram_tensor("ag", [128, PAD * 128], BF16, kind="ExternalInput").ap()
    xt_in = nc.dram_tensor("xt", [ntp, 128, 512], BF16, kind="ExternalInput").ap()
    cc_in = nc.dram_tensor("cc", [128, nt], F32, kind="ExternalInput").ap()
    d1_in = nc.dram_tensor("d1", [2, nt * 128], BF16, kind="ExternalInput").ap()
    w_in = nc.dram_tensor("wf", [128, 128], BF16, kind="ExternalInput").ap()
    w2_in = nc.dram_tensor("w2", [2, 256], BF16, kind="ExternalInput").ap()
    io_in = nc.dram_tensor("io", [128, 512], F32, kind="ExternalInput").ap()
    st_out = nc.dram_tensor("stage", [nsg, 128, GRP * 256], stage_dt,
                            kind="ExternalOutput").ap()
    pl_out = nc.dram_tensor("pool", [2, 128, 512], F32, kind="ExternalOutput").ap()

    relu = mybir.ActivationFunctionType.Relu
    iseq = mybir.AluOpType.is_equal

    with tile.TileContext(nc) as tc:
        with tc.tile_pool(name="consts", bufs=1) as consts, \
             tc.tile_pool(name="ep", bufs=3) as ep_pool, \
             tc.tile_pool(name="stg", bufs=3) as stg_pool, \
             tc.tile_pool(name="xp", bufs=4) as x_pool, \
             tc.tile_pool(name="fine", bufs=3) as fine_pool, \
             tc.tile_pool(name="ohn", bufs=3) as ohn_pool, \
             tc.tile_pool(name="pout", bufs=1) as pout_pool, \
             tc.tile_pool(name="eps", bufs=4, space="PSUM") as eps_pool, \
             tc.tile_pool(name="fps", bufs=2, space="PSUM") as fps_pool, \
             tc.tile_pool(name="pps", bufs=1, space="PSUM") as pps_pool:

            agt = consts.tile([128, PAD * 128], BF16, tag="agt")
            nc.sync.dma_start(agt[:], ag_in[:])
            w128 = consts.tile([128, 128], BF16, tag="w128")
            nc.sync.dma_start(w128[:], w_in[:])
            w2b2 = consts.tile([2, 256], BF16, tag="w2b2")
            nc.sync.dma_start(w2b2[:], w2_in[:])
            iot = consts.tile([128, 512], F32, tag="iot")
            nc.sync.dma_start(iot[:], io_in[:])
            cct = consts.tile([128, nt], F32, tag="cct")
            nc.sync.dma_start(cct[:], cc_in[:])
            d1t = consts.tile([2, nt * 128], BF16, tag="d1t")
            nc.sync.dma_start(d1t[:], d1_in[:])

            pool_ps = [pps_pool.tile([128, 512], F32, tag=f"pp{b}", name=f"pool_ps{b}")
                       for b in range(2)]

            def edge_group(grp):
                ept = ep_pool.tile([128, GRP * 1024], BF16, tag="ept",
                                   name=f"ept{grp}")
                nc.sync.dma_start(ept[:], ep_in[grp])
                stg = stg_pool.tile([128, GRP * 256], stage_dt, tag="stg",
                                    name=f"stg{grp}")
                for g in range(GRP):
                    eps = eps_pool.tile([128, 256], F32, tag="eps",
                                        name=f"eps{grp}_{g}")
                    for j in range(PAD):
                        nc.tensor.matmul(
                            eps[:], agt[:, j * 128:(j + 1) * 128],
                            ept[:, g * 1024 + j * 256: g * 1024 + (j + 1) * 256],
                            start=(j == 0), stop=(j == PAD - 1))
                    dst = stg[:, g * 256:(g + 1) * 256]
                    if g % 2 == 0:
                        nc.vector.tensor_copy(dst, eps[:])
                    else:
                        nc.scalar.copy(dst, eps[:])
                nc.sync.dma_start(st_out[grp], stg[:])

            def node_pair(tp):
                xtt = x_pool.tile([128, 512], BF16, tag="xt", name=f"xt{tp}")
                nc.sync.dma_start(xtt[:], xt_in[tp])
                for half in range(2):
                    t = tp * 2 + half
                    ohn = ohn_pool.tile([128, 512], BF16, tag="ohn", name=f"ohn{t}")
                    nc.vector.tensor_scalar(ohn[:], iot[:], cct[:, t:t + 1], None, iseq)
                    fps = fps_pool.tile([128, 256], F32, tag="fps", name=f"fps{t}")
                    # d-term + bias first (writes all elements), then x@W halves
                    nc.tensor.matmul(fps[:], d1t[:, t * 128:(t + 1) * 128], w2b2[:],
                                     start=True, stop=False)
                    for bch in range(2):
                        nc.tensor.matmul(
                            fps[:, bch * 128:(bch + 1) * 128],
                            xtt[:, bch * 256 + half * 128: bch * 256 + (half + 1) * 128],
                            w128[:], start=False, stop=True)
                    fsb = fine_pool.tile([128, 256], BF16, tag="fsb", name=f"fsb{t}")
                    nc.scalar.activation(fsb[:], fps[:], relu)
                    for bch in range(2):
                        nc.tensor.matmul(pool_ps[bch][:],
                                         fsb[:, bch * 128:(bch + 1) * 128], ohn[:],
                                         start=(t == 0), stop=(t == nt - 1))

            for i in range(max(nsg, ntp)):
                if i < nsg:
                    edge_group(i)
                if i < ntp:
                    node_pair(i)

            for bch in range(2):
                po = pout_pool.tile([128, 512], F32, tag=f"po{bch}", name=f"po{bch}")
                nc.vector.tensor_copy(po[:], pool_ps[bch][:])
                nc.sync.dma_start(pl_out[bch], po[:])

    nc.compile()
    return nc


def kernel(x, edge_index, edge_attr, scale=None, closest_centroid_indices=None,
           distances=None, W=None, b=None, **_unused):
    global LAST_EXEC_NS
    x_np = np.asarray(x, dtype=np.float32)
    ei = np.asarray(edge_index)
    ea = np.asarray(edge_attr, dtype=np.float32)
    cci_in = closest_centroid_indices
    cci = np.asarray(cci_in).astype(np.int64)
    dist = np.asarray(distances, dtype=np.float32)
    W_np = np.asarray(W, dtype=np.float32)
    b_np = np.asarray(b, dtype=np.float32)

    B, N, H = x_np.shape
    E = ei.shape[1]
    assert H == 128 and B == 2 and N % N_CORES == 0

    # ---------------- host index prep ----------------
    em = _edge_prep(ei.astype(np.int64), cci, E)
    S_sub = em["S_sub"]
    per_grp = GRP * 128                                 # sub-groups per DMA group
    nsg = max((S_sub + N_CORES * per_grp - 1) // (N_CORES * per_grp), 1)
    spc = nsg * per_grp                                 # sub-groups per core
    npc = N // N_CORES
    nt = (npc + 127) // 128
    if nt % 2:
        nt += 1
    ntp = nt // 2
    npc_pad = nt * 128

    ea_ext = np.concatenate([ea, np.zeros((2, 1, H), np.float32)], 1)

    iota512 = np.broadcast_to(np.arange(512, dtype=np.float32), (128, 512)).copy()
    # A_j[s, e] = 1 iff s == j*32 + e//PAD; stored as A_j^T [e, s]
    e_over = np.arange(128)[:, None] // PAD
    aseg = np.concatenate([(e_over + j * SUBS == np.arange(128)[None, :])
                           for j in range(PAD)], 1).astype(NPBF16)  # [128,PAD*128]
    wfull = W_np[:128].astype(NPBF16)                            # [128,128]
    w2row = np.concatenate([W_np[128][None, :], b_np[None, :]], 0)  # [2,128]
    w2b2 = np.concatenate([w2row, w2row], 1).astype(NPBF16)      # [2,256]

    in_maps = []
    for c in range(N_CORES):
        lo_s, hi_s = c * spc, min((c + 1) * spc, S_sub)
        gath = np.full((spc, PAD), E, np.int64)
        if hi_s > lo_s:
            gath[:hi_s - lo_s] = em["gath"][lo_s:hi_s]
        g = ea_ext[:, gath.reshape(-1), :]              # [2, spc*PAD, 128]
        g = g.reshape(2, nsg, GRP, PAD, SUBS, PAD, 128).transpose(1, 4, 5, 2, 3, 0, 6)
        ep = np.ascontiguousarray(g.reshape(nsg, 128, GRP * 1024)).astype(NPBF16)

        lo = c * npc
        xs = np.zeros((2, npc_pad, 128), np.float32)
        xs[:, :npc] = x_np[:, lo:lo + npc]
        xt = np.ascontiguousarray(
            xs.reshape(2, ntp, 256, 128).transpose(1, 3, 0, 2).reshape(ntp, 128, 512)
        ).astype(NPBF16)
        cc_pad = np.full(npc_pad, -1.0, np.float32)
        cc_pad[:npc] = cci[lo:lo + npc].astype(np.float32)
        cc = np.ascontiguousarray(cc_pad.reshape(nt, 128).T)    # [128, nt]
        d1 = np.ones((2, npc_pad), np.float32)
        d1[0, :] = 0.0
        d1[0, :npc] = dist[lo:lo + npc]

        in_maps.append({"ep": ep, "ag": aseg, "xt": xt, "cc": cc,
                        "d1": d1.astype(NPBF16), "wf": wfull, "w2": w2b2,
                        "io": iota512})

    # ---------------- build + run ----------------
    nc = _build_program(nsg, ntp, nt)
    trace = os.environ.get("BASS_KERNEL_TRACE", "0") == "1"
    if trace:
        try:
            import profhook
            profhook.install()
        except Exception:
            trace = False
    res = run_bass_kernel_spmd(nc, in_maps, core_ids=list(range(N_CORES)),
                               trace=trace)
    LAST_EXEC_NS = res.exec_time_ns

    # ---------------- host postprocessing ----------------
    pool_sum = np.zeros((2, 128, 512), np.float64)
    for c in range(N_CORES):
        pool_sum += res.results[c]["pool"]
    pool_sum = pool_sum.astype(np.float32)
    ncnt = np.bincount(cci, minlength=N_CENTROIDS).astype(np.float32)
    node_avg = pool_sum.transpose(0, 2, 1) / np.clip(ncnt, 1.0, None)[None, :, None]

    # coarse edge attrs: gather real sub-group rows, reduce per slot
    U = em["U"]
    chunks = []
    for c in range(N_CORES):
        lo_s, hi_s = c * spc, min((c + 1) * spc, S_sub)
        if hi_s > lo_s:
            st = res.results[c]["stage"].astype(np.float32)
            st = st.reshape(nsg, 128, GRP, 256).transpose(0, 2, 1, 3).reshape(spc, 256)
            chunks.append(st[:hi_s - lo_s])
    out_attr = np.zeros((2, E_COARSE + 1, 128), np.float32)
    if U > 0:
        rows = np.concatenate(chunks, 0)                         # [S_sub, 256]
        esums = np.add.reduceat(rows, em["sub_lo"][:U], axis=0)  # [U, 256]
        emeans = esums / np.clip(em["counts"].astype(np.float32), 1.0, None)[:, None]
        out_attr[0, :U] = emeans[:, :128]
        out_attr[1, :U] = emeans[:, 128:]

    uniq = em["uniq"]
    uniq_pad = np.full(E_COARSE + 1, SENTINEL, np.int64)
    uniq_pad[:min(len(uniq), E_COARSE + 1)] = uniq[:E_COARSE + 1]
    idx_dt = np.asarray(cci_in).dtype
    if idx_dt.kind not in "iu":
        idx_dt = np.dtype(np.int32)
    ce = np.where(uniq_pad < SENTINEL,
                  np.stack([uniq_pad // N_CENTROIDS, uniq_pad % N_CENTROIDS]),
                  -1).astype(idx_dt)

    return (node_avg, out_attr, ce, np.asarray(cci_in), np.asarray(distances))


# revision 10
# speedup vs baseline: 2.3459x; 1.3853x over previous
"""Trainium2 Bass kernel for the GNN coarsening layer (nn_Coarse_layer).

Pipeline (B=2 batches, N=100k nodes, E=800k edges, H=128, C=512 centroids):
  1. fine = relu(concat([x, dist]) @ W + b)                      [B,N,H]
  2. node_avg = segment_mean(fine, cci, C)                       [B,C,H]
  3. coarse edges: group edges by sorted centroid-pair key, drop
     intra-centroid edges, unique keys -> segment_mean(edge_attr) [B,Ec+1,H]

Strategy: the host does the cheap integer index work — pair keys, unique,
argsort — and pads each coarse-edge group to a multiple of PAD=4 edges
(null edges point at an appended all-zero row).  With that layout the big
segment-sum becomes a constant-stationary matmul: fixed shifted aggregation
matrices A_j^T[128, 128] (A_j[s, e] = 1 iff s == j*32 + e//4) reduce 128
streamed edge rows into 32 partial-group rows each, all four subtiles
accumulating into one PSUM tile, with both batches side by side in the
moving operand.  The tensor engine streams with zero per-tile setup work,
PSUM accumulates in fp32, and the host divides by group counts and sums the
split sub-groups (np.add.reduceat) while assembling the output.  The tiny
MLP + 512-way node pooling run on-device as bf16 matmuls with a per-tile
iota/is_equal one-hot for the pooling reduction.  DMA traffic is batched:
GRP=4 superbins ride one 1MB load and one staging store.
"""

import os
import sys

for _p in ("/opt/trn_rl_repo", "/root/.axon_site/_ro/trn_rl_repo"):
    if os.path.isdir(_p) and _p not in sys.path:
        sys.path.append(_p)

import numpy as np
import ml_dtypes

import concourse.bacc as bacc
import concourse.tile as tile
import concourse.mybir as mybir
from concourse.bass_utils import run_bass_kernel_spmd

F32 = mybir.dt.float32
BF16 = mybir.dt.bfloat16
NPBF16 = ml_dtypes.bfloat16

N_CENTROIDS = 512
E_COARSE = N_CENTROIDS * (N_CENTROIDS - 1) // 2
SENTINEL = N_CENTROIDS * N_CENTROIDS
N_CORES = 8

PAD = 4                  # edges per sub-group (aggregation matrix row width)
SUBS = 128 // PAD        # sub-groups produced per matmul (32)
GRP = 4                  # superbins (128 sub-groups each) per DMA transfer
STAGE_F32 = False        # staging dtype; bf16 halves the store traffic

LAST_EXEC_NS = None      # filled when BASS_KERNEL_TRACE=1


def _edge_prep(edge_index, cci, E):
    """Sort edges by coarse-pair slot; pad each slot to a multiple of PAD."""
    sv = cci[edge_index[0]]
    ev = cci[edge_index[1]]
    vmin = np.minimum(sv, ev)
    vmax = np.maximum(sv, ev)
    keys = np.where(sv != ev, vmin * N_CENTROIDS + vmax, SENTINEL)
    uniq, inv = np.unique(keys, return_inverse=True)
    U = int(np.searchsorted(uniq, SENTINEL))        # valid (non-sentinel) slots
    order = np.argsort(inv, kind="stable")
    nval = int(np.searchsorted(inv[order], U))      # edges in valid slots
    order = order[:nval]
    counts = np.bincount(inv[order], minlength=max(U, 1))[:U]
    cum = np.zeros(U + 1, np.int64)
    cum[1:] = np.cumsum(counts)

    subcnt = (counts + PAD - 1) // PAD
    sub_lo = np.zeros(U + 1, np.int64)
    sub_lo[1:] = np.cumsum(subcnt)
    S_sub = int(sub_lo[-1])
    slot_of_sub = np.repeat(np.arange(U, dtype=np.int64), subcnt)
    within = np.arange(S_sub, dtype=np.int64) - sub_lo[slot_of_sub]
    epos = cum[slot_of_sub][:, None] + within[:, None] * PAD + np.arange(PAD)[None, :]
    ok = epos < cum[slot_of_sub + 1][:, None]
    epos_c = np.minimum(epos, max(nval - 1, 0))
    gath = np.where(ok, order[epos_c], E)           # E == appended zero row
    return dict(uniq=uniq, U=U, counts=counts, sub_lo=sub_lo, S_sub=S_sub,
                gath=gath)


def _build_program(nsg, ntp, nt):
    """One SPMD program; all per-core variation comes in through inputs."""
    nc = bacc.Bacc("TRN2", target_bir_lowering=False, debug=False,
                   enable_asserts=False, num_devices=N_CORES)
    stage_dt = F32 if STAGE_F32 else BF16
    ep_in = nc.dram_tensor("ep", [nsg, 128, GRP * 1024], BF16, kind="ExternalInput").ap()
    ag_in = nc.dram_tensor("ag", [128, PAD * 128], BF16, kind="ExternalInput").ap()
    xt_in = nc.dram_tensor("xt", [ntp, 128, 512], BF16, kind="ExternalInput").ap()
    cc_in = nc.dram_tensor("cc", [128, nt], F32, kind="ExternalInput").ap()
    d1_in = nc.dram_tensor("d1", [2, nt * 128], BF16, kind="ExternalInput").ap()
    w_in = nc.dram_tensor("wf", [128, 128], BF16, kind="ExternalInput").ap()
    w2_in = nc.dram_tensor("w2", [2, 256], BF16, kind="ExternalInput").ap()
    io_in = nc.dram_tensor("io", [128, 512], F32, kind="ExternalInput").ap()
    st_out = nc.dram_tensor("stage", [nsg, 128, GRP * 256], stage_dt,
                            kind="ExternalOutput").ap()
    pl_out = nc.dram_tensor("pool", [2, 128, 512], F32, kind="ExternalOutput").ap()

    relu = mybir.ActivationFunctionType.Relu
    iseq = mybir.AluOpType.is_equal

    with tile.TileContext(nc) as tc:
        with tc.tile_pool(name="consts", bufs=1) as consts, \
             tc.tile_pool(name="ep", bufs=4) as ep_pool, \
             tc.tile_pool(name="stg", bufs=4) as stg_pool, \
             tc.tile_pool(name="xp", bufs=4) as x_pool, \
             tc.tile_pool(name="fine", bufs=4) as fine_pool, \
             tc.tile_pool(name="ohn", bufs=4) as ohn_pool, \
             tc.tile_pool(name="pout", bufs=1) as pout_pool, \
             tc.tile_pool(name="eps", bufs=4, space="PSUM") as eps_pool, \
             tc.tile_pool(name="fps", bufs=2, space="PSUM") as fps_pool, \
             tc.tile_pool(name="pps", bufs=1, space="PSUM") as pps_pool:

            agt = consts.tile([128, PAD * 128], BF16, tag="agt")
            nc.sync.dma_start(agt[:], ag_in[:])
            w128 = consts.tile([128, 128], BF16, tag="w128")
            nc.sync.dma_start(w128[:], w_in[:])
            w2b2 = consts.tile([2, 256], BF16, tag="w2b2")
            nc.sync.dma_start(w2b2[:], w2_in[:])
            iot = consts.tile([128, 512], F32, tag="iot")
            nc.sync.dma_start(iot[:], io_in[:])
            cct = consts.tile([128, nt], F32, tag="cct")
            nc.sync.dma_start(cct[:], cc_in[:])
            d1t = consts.tile([2, nt * 128], BF16, tag="d1t")
            nc.sync.dma_start(d1t[:], d1_in[:])

            pool_ps = [pps_pool.tile([128, 512], F32, tag=f"pp{b}", name=f"pool_ps{b}")
                       for b in range(2)]

            def edge_group(grp):
                ept = ep_pool.tile([128, GRP * 1024], BF16, tag="ept",
                                   name=f"ept{grp}")
                nc.sync.dma_start(ept[:], ep_in[grp])
                stg = stg_pool.tile([128, GRP * 256], stage_dt, tag="stg",
                                    name=f"stg{grp}")
                for g in range(GRP):
                    eps = eps_pool.tile([128, 256], F32, tag="eps",
                                        name=f"eps{grp}_{g}")
                    for j in range(PAD):
                        nc.tensor.matmul(
                            eps[:], agt[:, j * 128:(j + 1) * 128],
                            ept[:, g * 1024 + j * 256: g * 1024 + (j + 1) * 256],
                            start=(j == 0), stop=(j == PAD - 1))
                    dst = stg[:, g * 256:(g + 1) * 256]
                    if g % 2 == 0:
                        nc.vector.tensor_copy(dst, eps[:])
                    else:
                        nc.scalar.copy(dst, eps[:])
                nc.gpsimd.dma_start(st_out[grp], stg[:])

            def node_pair(tp):
                xtt = x_pool.tile([128, 512], BF16, tag="xt", name=f"xt{tp}")
                nc.sync.dma_start(xtt[:], xt_in[tp])
                for half in range(2):
                    t = tp * 2 + half
                    ohn = ohn_pool.tile([128, 512], BF16, tag="ohn", name=f"ohn{t}")
                    nc.vector.tensor_scalar(ohn[:], iot[:], cct[:, t:t + 1], None, iseq)
                    fps = fps_pool.tile([128, 256], F32, tag="fps", name=f"fps{t}")
                    # d-term + bias first (writes all elements), then x@W halves
                    nc.tensor.matmul(fps[:], d1t[:, t * 128:(t + 1) * 128], w2b2[:],
                                     start=True, stop=False)
                    for bch in range(2):
                        nc.tensor.matmul(
                            fps[:, bch * 128:(bch + 1) * 128],
                            xtt[:, bch * 256 + half * 128: bch * 256 + (half + 1) * 128],
                            w128[:], start=False, stop=True)
                    fsb = fine_pool.tile([128, 256], BF16, tag="fsb", name=f"fsb{t}")
                    nc.scalar.activation(fsb[:], fps[:], relu)
                    for bch in range(2):
                        nc.tensor.matmul(pool_ps[bch][:],
                                         fsb[:, bch * 128:(bch + 1) * 128], ohn[:],
                                         start=(t == 0), stop=(t == nt - 1))

            for i in range(max(nsg, ntp)):
                if i < nsg:
                    edge_group(i)
                if i < ntp:
                    node_pair(i)

            for bch in range(2):
                po = pout_pool.tile([128, 512], F32, tag=f"po{bch}", name=f"po{bch}")
                nc.vector.tensor_copy(po[:], pool_ps[bch][:])
                nc.sync.dma_start(pl_out[bch], po[:])

    nc.compile()
    return nc


def kernel(x, edge_index, edge_attr, scale=None, closest_centroid_indices=None,
           distances=None, W=None, b=None, **_unused):
    global LAST_EXEC_NS
    x_np = np.asarray(x, dtype=np.float32)
    ei = np.asarray(edge_index)
    ea = np.asarray(edge_attr, dtype=np.float32)
    cci_in = closest_centroid_indices
    cci = np.asarray(cci_in).astype(np.int64)
    dist = np.asarray(distances, dtype=np.float32)
    W_np = np.asarray(W, dtype=np.float32)
    b_np = np.asarray(b, dtype=np.float32)

    B, N, H = x_np.shape
    E = ei.shape[1]
    assert H == 128 and B == 2 and N % N_CORES == 0

    # ---------------- host index prep ----------------
    em = _edge_prep(ei.astype(np.int64), cci, E)
    S_sub = em["S_sub"]
    per_grp = GRP * 128                                 # sub-groups per DMA group
    nsg = max((S_sub + N_CORES * per_grp - 1) // (N_CORES * per_grp), 1)
    spc = nsg * per_grp                                 # sub-groups per core
    npc = N // N_CORES
    nt = (npc + 127) // 128
    if nt % 2:
        nt += 1
    ntp = nt // 2
    npc_pad = nt * 128

    ea_ext = np.concatenate([ea, np.zeros((2, 1, H), np.float32)], 1)

    iota512 = np.broadcast_to(np.arange(512, dtype=np.float32), (128, 512)).copy()
    # A_j[s, e] = 1 iff s == j*32 + e//PAD; stored as A_j^T [e, s]
    e_over = np.arange(128)[:, None] // PAD
    aseg = np.concatenate([(e_over + j * SUBS == np.arange(128)[None, :])
                           for j in range(PAD)], 1).astype(NPBF16)  # [128,PAD*128]
    wfull = W_np[:128].astype(NPBF16)                            # [128,128]
    w2row = np.concatenate([W_np[128][None, :], b_np[None, :]], 0)  # [2,128]
    w2b2 = np.concatenate([w2row, w2row], 1).astype(NPBF16)      # [2,256]

    in_maps = []
    for c in range(N_CORES):
        lo_s, hi_s = c * spc, min((c + 1) * spc, S_sub)
        gath = np.full((spc, PAD), E, np.int64)
        if hi_s > lo_s:
            gath[:hi_s - lo_s] = em["gath"][lo_s:hi_s]
        g = ea_ext[:, gath.reshape(-1), :]              # [2, spc*PAD, 128]
        g = g.reshape(2, nsg, GRP, PAD, SUBS, PAD, 128).transpose(1, 4, 5, 2, 3, 0, 6)
        ep = np.ascontiguousarray(g.reshape(nsg, 128, GRP * 1024)).astype(NPBF16)

        lo = c * npc
        xs = np.zeros((2, npc_pad, 128), np.float32)
        xs[:, :npc] = x_np[:, lo:lo + npc]
        xt = np.ascontiguousarray(
            xs.reshape(2, ntp, 256, 128).transpose(1, 3, 0, 2).reshape(ntp, 128, 512)
        ).astype(NPBF16)
        cc_pad = np.full(npc_pad, -1.0, np.float32)
        cc_pad[:npc] = cci[lo:lo + npc].astype(np.float32)
        cc = np.ascontiguousarray(cc_pad.reshape(nt, 128).T)    # [128, nt]
        d1 = np.ones((2, npc_pad), np.float32)
        d1[0, :] = 0.0
        d1[0, :npc] = dist[lo:lo + npc]

        in_maps.append({"ep": ep, "ag": aseg, "xt": xt, "cc": cc,
                        "d1": d1.astype(NPBF16), "wf": wfull, "w2": w2b2,
                        "io": iota512})

    # ---------------- build + run ----------------
    nc = _build_program(nsg, ntp, nt)
    trace = os.environ.get("BASS_KERNEL_TRACE", "0") == "1"
    if trace:
        try:
            import profhook
            profhook.install()
        except Exception:
            trace = False
    res = run_bass_kernel_spmd(nc, in_maps, core_ids=list(range(N_CORES)),
                               trace=trace)
    LAST_EXEC_NS = res.exec_time_ns

    # ---------------- host postprocessing ----------------
    pool_sum = np.zeros((2, 128, 512), np.float64)
    for c in range(N_CORES):
        pool_sum += res.results[c]["pool"]
    pool_sum = pool_sum.astype(np.float32)
    ncnt = np.bincount(cci, minlength=N_CENTROIDS).astype(np.float32)
    node_avg = pool_sum.transpose(0, 2, 1) / np.clip(ncnt, 1.0, None)[None, :, None]

    # coarse edge attrs: gather real sub-group rows, reduce per slot
    U = em["U"]
    chunks = []
    for c in range(N_CORES):
        lo_s, hi_s = c * spc, min((c + 1) * spc, S_sub)
        if hi_s > lo_s:
            st = res.results[c]["stage"].astype(np.float32)
            st = st.reshape(nsg, 128, GRP, 256).transpose(0, 2, 1, 3).reshape(spc, 256)
            chunks.append(st[:hi_s - lo_s])
    out_attr = np.zeros((2, E_COARSE + 1, 128), np.float32)
    if U > 0:
        rows = np.concatenate(chunks, 0)                         # [S_sub, 256]
        esums = np.add.reduceat(rows, em["sub_lo"][:U], axis=0)  # [U, 256]
        emeans = esums / np.clip(em["counts"].astype(np.float32), 1.0, None)[:, None]
        out_attr[0, :U] = emeans[:, :128]
        out_attr[1, :U] = emeans[:, 128:]

    uniq = em["uniq"]
    uniq_pad = np.full(E_COARSE + 1, SENTINEL, np.int64)
    uniq_pad[:min(len(uniq), E_COARSE + 1)] = uniq[:E_COARSE + 1]
    idx_dt = np.asarray(cci_in).dtype
    if idx_dt.kind not in "iu":
        idx_dt = np.dtype(np.int32)
    ce = np.where(uniq_pad < SENTINEL,
                  np.stack([uniq_pad // N_CENTROIDS, uniq_pad % N_CENTROIDS]),
                  -1).astype(idx_dt)

    return (node_avg, out_attr, ce, np.asarray(cci_in), np.asarray(distances))
